# revision 11
# baseline (speedup 1.0000x reference)
"""GCN block (adj @ x @ W -> masked BatchNorm(train) -> relu) on 8 TRN2 cores.

Sharding: data-parallel over the batch dim, 8 graphs per core. Host-side
packing (our chosen input layout, applied to the full inputs):
  * adj rows are pre-scaled by the node mask (row scaling commutes with the
    matmul chain, and masked BN stats need the masked product anyway), then
    transposed so the contraction dim m lands on SBUF partitions.
  * adjT_masked and x are packed into one per-pair-of-graphs "blob" so each
    load is a single large DMA (the toolchain encodes at most ONE semaphore
    wait per instruction, so every matmul must have a single upstream DMA).

Per-core device pipeline:
  chain1 (per graph):  tT[d, n] = sum_m x[m, d] * adjTm[m, n]      (f32r PE)
  chain2 (per graph):  OT[e, n] = sum_d W[d, e] * tT[d, n]         (f32r PE)
  bn_stats/bn_aggr over the free (n) axis of OT in PSUM -> per-core
  (sum, sumsq)[e]; 2 KB AllReduce across the 8 cores; then
  scale[e] = gamma*rsqrt(var+eps), shift[e] = beta - mean*scale,
  W' = W * scale (via a PE outer-product broadcast of scale),
  out[n, e] = relu(mask[n] * (sum_d tT[d,n]*W'[d,e] + shift[e]))   (f32r PE
  + one K=1 matmul adding the shift row + one ACT relu with per-partition
  mask scale), stored via SWDGE so loads (HWDGE) and stores (SWDGE) each
  use every DMA semaphore lane at most once.
"""

import numpy as np

import concourse.bass as bass
import concourse.mybir as mybir
import concourse.tile as tile
from concourse.bass_utils import run_bass_kernel_spmd
from concourse.vector_clock import ScopedClock, VectorClock

B, N, DIN, DOUT = 64, 512, 256, 256
EPS = 1e-5
NCORES = 8
GPC = B // NCORES          # graphs per core
NPAIR = GPC // 2           # paired loads/stores
P = 128
NC_N = N // P              # 4
NC_M = N // P              # 4
NC_D = DIN // P            # 2
NC_E = DOUT // P           # 2

f32 = mybir.dt.float32
f32r = mybir.dt.float32r
bf16 = mybir.dt.bfloat16

# per-graph blob columns: adjT_masked [p, mc, n] then x [p, mc, d]
ADJW = NC_M * N            # 2048
XW = NC_M * DIN            # 1024
GBLOB = ADJW + XW          # 3072 per graph
BLOBW = 2 * GBLOB          # per pair

# aux columns
W0 = 0                         # W packed [p, dc, e] -> 512 cols
IDENT0 = W0 + NC_D * DOUT      # 512
ONES0 = IDENT0 + P             # 640 (128 cols of 1.0; row 0 used as ones-row)
GAMMA0 = ONES0 + P             # 768
BETA0 = GAMMA0 + NC_E          # 770
MASKT0 = BETA0 + NC_E          # 772 (maskT[p, g*4+c] = mask[b, c*128+p])
INVN0 = MASKT0 + GPC * NC_N    # 804
EPS0 = INVN0 + 1               # 805
AUXW = EPS0 + 1                # 806

ActFn = mybir.ActivationFunctionType
Alu = mybir.AluOpType


class _TileContext1W(tile.TileContext):
    """Split the tail drain's multi-waits into single-wait sequencer nops
    (this walrus build encodes at most one sync wait per instruction)."""

    def _drain_and_barrier(self, tick_clock, wait_clock):
        gc = tick_clock.global_clock
        n = len(gc)
        for p in range(n):
            t = gc[p]
            if t > 0:
                single = VectorClock([t if i == p else 0 for i in range(n)])
                nop = self.nc.sync.nop(nofuse=True, hint=f"drain_split_{p}")
                wait_clock.add_sem_waits(nop.ins, ScopedClock({None: single}))
        self.nc.sync.drain()
        self.nc.all_engine_barrier()
        assert self.sems is not None
        popped = self.nc._tile_sem_poison_stack.pop()
        assert popped is self._sem_poison
        self.nc.clear_and_free_semaphores(list(self.sems.allocated().values()))
        self.nc.all_engine_barrier()


def _build_nc():
    nc = bass.Bass(num_devices=NCORES)
    blob_d = nc.dram_tensor("blob", [NPAIR, P, BLOBW], f32r, kind="ExternalInput")
    aux_d = nc.dram_tensor("aux", [P, AUXW], f32r, kind="ExternalInput")
    out_d = nc.dram_tensor("out", [GPC, N, DOUT], f32, kind="ExternalOutput")

    with _TileContext1W(nc) as tc:
        with (
            tc.tile_pool(name="aux_p", bufs=1) as aux_p,
            tc.tile_pool(name="blob_p", bufs=NPAIR) as blob_p,
            tc.tile_pool(name="tT_p", bufs=GPC) as tT_p,
            tc.tile_pool(name="row_p", bufs=1) as row_p,
            tc.tile_pool(name="o_p", bufs=NPAIR) as o_p,
            tc.tile_pool(name="st_p", bufs=1) as st_p,
            tc.tile_pool(name="dram", bufs=2, space="DRAM") as dram_p,
        ):
            aux = aux_p.tile([P, AUXW], f32r)
            nc.sync.dma_start(out=aux, in_=aux_d[:, :])
            auxf = aux.bitcast(f32)
            ident_f32 = auxf[:, IDENT0:IDENT0 + P]
            ones_row = aux[0:1, ONES0:ONES0 + P]
            gamma_ap = auxf[:, GAMMA0:GAMMA0 + NC_E]
            beta_ap = auxf[:, BETA0:BETA0 + NC_E]
            invn_ap = auxf[:, INVN0:INVN0 + 1]
            eps_ap = auxf[:, EPS0:EPS0 + 1]

            tT_tiles = []
            osb_tiles = []

            with (
                tc.tile_pool(name="ps_g1", bufs=1, space="PSUM") as ps_g1,
                tc.tile_pool(name="ps_tT", bufs=2, space="PSUM") as ps_tT,
                tc.tile_pool(name="ps_ot", bufs=1, space="PSUM") as ps_ot,
            ):
                # observer gadgets: absorb the aux-DMA wait on PE/ACT/DVE
                g1 = ps_g1.tile([1, 1], f32)
                nc.tensor.matmul(
                    g1[:, :], auxf[0:1, ONES0:ONES0 + 1],
                    auxf[0:1, ONES0:ONES0 + 1], start=True, stop=True,
                )
                gsc = st_p.tile([P, 2], f32, tag="gadget")
                nc.scalar.copy(out=gsc[:, 0:1], in_=eps_ap)
                nc.vector.tensor_copy(out=gsc[:, 1:2], in_=invn_ap)
                # read g1 so its PSUM bank is reader-released before recycling
                gr1 = st_p.tile([1, 1], f32, tag="gadget3")
                nc.vector.tensor_copy(out=gr1, in_=g1[:, :])

                st = st_p.tile([P, NC_E, GPC, 6], f32)

                for pair in range(NPAIR):
                    blob = blob_p.tile([P, BLOBW], f32r)
                    nc.sync.dma_start(out=blob, in_=blob_d[pair, :, :])
                    for h in range(2):
                        g = 2 * pair + h
                        off = h * GBLOB
                        # chain1: tT[d, n] = sum_m x[m, d] * adjTm[m, n]
                        tT_ps = ps_tT.tile([P, NC_D, N], f32, tag="tT")
                        for dc in range(NC_D):
                            for kc in range(NC_M):
                                nc.tensor.matmul(
                                    tT_ps[:, dc, :],
                                    blob[:, off + ADJW + kc * DIN + dc * P:
                                         off + ADJW + kc * DIN + (dc + 1) * P],
                                    blob[:, off + kc * N:off + (kc + 1) * N],
                                    start=(kc == 0), stop=(kc == NC_M - 1),
                                )
                        tT = tT_p.tile([P, NC_D, N], f32r)
                        for dc in range(NC_D):
                            nc.vector.tensor_copy(
                                out=tT[:, dc, :], in_=tT_ps[:, dc, :])
                        tT_tiles.append(tT)

                        # chain2: OT[e, n] = sum_d W[d, e] * tT[d, n]
                        ot_ps = ps_ot.tile([P, NC_E, N], f32, tag="ot")
                        for ec in range(NC_E):
                            for dc in range(NC_D):
                                nc.tensor.matmul(
                                    ot_ps[:, ec, :],
                                    aux[:, dc * DOUT + ec * P:
                                        dc * DOUT + (ec + 1) * P],
                                    tT[:, dc, :],
                                    start=(dc == 0), stop=(dc == NC_D - 1),
                                )
                        # masked stats straight off PSUM (free axis = n)
                        for ec in range(NC_E):
                            nc.vector.bn_stats(
                                out=st[:, ec, g, :], in_=ot_ps[:, ec, :])

                # --- stats -> (sum, sumsq) -> AllReduce ---
                mv = st_p.tile([P, NC_E, 2], f32)
                for ec in range(NC_E):
                    nc.vector.bn_aggr(out=mv[:, ec, :], in_=st[:, ec, :, :])
                cnt = float(GPC * N)  # bn count per core (incl. masked zeros)
                pack = st_p.tile([P, 2 * NC_E], f32)
                for ec in range(NC_E):
                    nc.vector.tensor_scalar_mul(
                        out=pack[:, ec:ec + 1], in0=mv[:, ec, 0:1], scalar1=cnt)
                    nc.vector.tensor_scalar(
                        out=pack[:, NC_E + ec:NC_E + ec + 1],
                        in0=mv[:, ec, 0:1],
                        scalar1=mv[:, ec, 0:1], scalar2=mv[:, ec, 1:2],
                        op0=Alu.mult, op1=Alu.add,
                    )
                    nc.vector.tensor_scalar_mul(
                        out=pack[:, NC_E + ec:NC_E + ec + 1],
                        in0=pack[:, NC_E + ec:NC_E + ec + 1], scalar1=cnt)

                ar_in = dram_p.tile([P, 2 * NC_E], f32)
                ar_out = dram_p.tile([P, 2 * NC_E], f32)
                nc.sync.dma_start(out=ar_in[:, :], in_=pack)
                nc.gpsimd.collective_compute(
                    "AllReduce", Alu.add,
                    replica_groups=[list(range(NCORES))],
                    ins=[ar_in[:, :].opt()],
                    outs=[ar_out[:, :].opt()],
                )
                sq = st_p.tile([P, 2 * NC_E], f32)
                nc.sync.dma_start(out=sq, in_=ar_out[:, :])

            with (
                tc.tile_pool(name="ps_w", bufs=3, space="PSUM") as ps_w,
                tc.tile_pool(name="ps2", bufs=2, space="PSUM") as ps2,
            ):
                # PE observers: a psum-free ldweights absorbs the pre-AR
                # DVE tick; three aux-reading matmuls touch the recycled ps_w
                # banks (each needs the pool-transition engine tick exactly
                # once); DVE then reads them so later writers are gated by
                # reader-completion (a single already-covered wait) instead.

                # --- scale/shift (all [128, NC_E], e on partitions) ---
                mean = st_p.tile([P, NC_E], f32)
                var = st_p.tile([P, NC_E], f32)
                m2 = st_p.tile([P, NC_E], f32)
                sd = st_p.tile([P, NC_E], f32)
                rs = st_p.tile([P, NC_E], f32)
                scale = st_p.tile([P, NC_E], f32)
                shift = st_p.tile([P, NC_E], f32)
                nc.vector.tensor_scalar_mul(out=mean, in0=sq[:, 0:NC_E],
                                            scalar1=invn_ap)
                nc.vector.tensor_scalar_mul(out=var, in0=sq[:, NC_E:2 * NC_E],
                                            scalar1=invn_ap)
                nc.vector.tensor_mul(out=m2, in0=mean, in1=mean)
                nc.vector.tensor_sub(out=var, in0=var, in1=m2)
                nc.scalar.activation(out=sd, in_=var, func=ActFn.Sqrt,
                                     bias=eps_ap, scale=1.0)
                nc.vector.reciprocal(out=rs, in_=sd)
                nc.vector.tensor_mul(out=scale, in0=rs, in1=gamma_ap)
                nc.vector.tensor_mul(out=m2, in0=mean, in1=scale)
                nc.vector.tensor_sub(out=shift, in0=beta_ap, in1=m2)
                # rows: scale/shift transposed to [1, DOUT]
                # (psum-free ldweights first: absorb the DVE tick so the
                #  recycled-bank write carries only the PE-engine wait)
                nc.tensor.ldweights(weights=scale[0:1, 0:1].bitcast(bf16))
                rows_ps = ps_w.tile([1, 2, DOUT], f32, tag="w")
                for ec in range(NC_E):
                    nc.tensor.transpose(
                        rows_ps[:, 0, ec * P:(ec + 1) * P],
                        scale[:, ec:ec + 1], ident_f32)
                    nc.tensor.transpose(
                        rows_ps[:, 1, ec * P:(ec + 1) * P],
                        shift[:, ec:ec + 1], ident_f32)
                rows = row_p.tile([1, 2, DOUT], f32r)
                nc.vector.tensor_copy(out=rows[:, 0, :], in_=rows_ps[:, 0, :])
                nc.vector.tensor_copy(out=rows[:, 1, :], in_=rows_ps[:, 1, :])
                # scale broadcast [128, DOUT] via ones-col x scale-row
                nc.tensor.ldweights(weights=rows[0:1, 1, 0:1].bitcast(bf16))
                scbc_ps = ps_w.tile([P, DOUT], f32, tag="w")
                nc.tensor.matmul(scbc_ps[:, :], ones_row, rows[:, 0, :],
                                 start=True, stop=True)
                scbc = row_p.tile([P, DOUT], f32)
                nc.vector.tensor_copy(out=scbc, in_=scbc_ps[:, :])
                wp = row_p.tile([P, NC_D, DOUT], f32r)
                for dc in range(NC_D):
                    nc.vector.tensor_mul(
                        out=wp[:, dc, :],
                        in0=auxf[:, dc * DOUT:(dc + 1) * DOUT], in1=scbc)

                for g in range(GPC):
                    if g >= 2:
                        # dummy ldweights: absorb the ACT tick (relu of g-2)
                        # before this graph's PSUM-slot-recycling matmuls
                        nc.tensor.ldweights(
                            weights=osb_tiles[g - 2][0:1, 0, 0, 0:64]
                            .bitcast(bf16))
                    tT = tT_tiles[g]
                    z_ps = ps2.tile([P, NC_N, DOUT], f32, tag="z")
                    for j in range(NC_N):
                        nc.tensor.matmul(
                            z_ps[:, j, :], ones_row, rows[:, 1, :],
                            start=True, stop=False,
                        )
                        for dc in range(NC_D):
                            nc.tensor.matmul(
                                z_ps[:, j, :],
                                tT[:, dc, j * P:(j + 1) * P],
                                wp[:, dc, :],
                                start=False, stop=(dc == NC_D - 1),
                            )
                    if g % 2 == 0:
                        osb = o_p.tile([P, 2, NC_N, DOUT], f32, tag="osb")
                    else:
                        osb = osb_tiles[g - 1]
                    osb_tiles.append(osb)
                    for j in range(NC_N):
                        nc.scalar.activation(
                            out=osb[:, g % 2, j, :], in_=z_ps[:, j, :],
                            func=ActFn.Relu, bias=0.0,
                            scale=auxf[:, MASKT0 + g * NC_N + j:
                                       MASKT0 + g * NC_N + j + 1],
                        )
                    if g % 2 == 1:
                        pair = g // 2
                        nc.gpsimd.dma_start(
                            out=out_d[2 * pair:2 * pair + 2]
                                .rearrange("g (c p) e -> p g c e", p=P),
                            in_=osb,
                        )
    return nc


_CACHE = {}


def _get_nc():
    if "nc" not in _CACHE:
        _CACHE["nc"] = _build_nc()
    return _CACHE["nc"]


def kernel(x, adj, mask, weight, bias, gamma, beta):
    x = np.asarray(x, dtype=np.float32)
    adj = np.asarray(adj, dtype=np.float32)
    mask = np.asarray(mask, dtype=np.float32)
    weight = np.asarray(weight, dtype=np.float32)
    gamma = np.asarray(gamma, dtype=np.float32)
    beta = np.asarray(beta, dtype=np.float32)
    # bias cancels exactly in train-mode batchnorm (the mean absorbs it).

    n_tot = float(mask.sum())
    inv_n = np.float32(1.0 / n_tot)

    w_pack = weight.reshape(NC_D, P, DOUT).transpose(1, 0, 2).reshape(P, NC_D * DOUT)
    ident = np.eye(P, dtype=np.float32)
    gam = gamma.reshape(NC_E, P).T.copy()
    bet = beta.reshape(NC_E, P).T.copy()

    in_maps = []
    for c in range(NCORES):
        bs = slice(c * GPC, (c + 1) * GPC)
        adjm = adj[bs] * mask[bs][:, :, None]          # [GPC, n, m]
        adjT = adjm.transpose(0, 2, 1)                 # [GPC, m, n]
        blob_adj = adjT.reshape(GPC, NC_M, P, N).transpose(0, 2, 1, 3) \
                       .reshape(GPC, P, ADJW)
        blob_x = x[bs].reshape(GPC, NC_M, P, DIN).transpose(0, 2, 1, 3) \
                      .reshape(GPC, P, XW)
        blob = np.concatenate([blob_adj, blob_x], axis=2)   # [GPC, P, GBLOB]
        blob = blob.reshape(NPAIR, 2, P, GBLOB).transpose(0, 2, 1, 3) \
                   .reshape(NPAIR, P, BLOBW)

        maskT = mask[bs].reshape(GPC, NC_N, P).transpose(2, 0, 1) \
                        .reshape(P, GPC * NC_N)
        aux = np.empty((P, AUXW), dtype=np.float32)
        aux[:, W0:W0 + NC_D * DOUT] = w_pack
        aux[:, IDENT0:IDENT0 + P] = ident
        aux[:, ONES0:ONES0 + P] = 1.0
        aux[:, GAMMA0:GAMMA0 + NC_E] = gam
        aux[:, BETA0:BETA0 + NC_E] = bet
        aux[:, MASKT0:MASKT0 + GPC * NC_N] = maskT
        aux[:, INVN0] = inv_n
        aux[:, EPS0] = np.float32(EPS)
        in_maps.append(dict(blob=np.ascontiguousarray(blob, dtype=np.float32),
                            aux=np.ascontiguousarray(aux)))

    nc = _get_nc()
    res = run_bass_kernel_spmd(nc, in_maps, core_ids=list(range(NCORES)))
    out = np.concatenate([r["out"] for r in res.results], axis=0)
    return out.reshape(B, N, DOUT)


# revision 22
# speedup vs baseline: 33516.9045x; 33516.9045x over previous
"""GCN block (adj @ x @ W -> masked BatchNorm(train) -> relu) on 8 TRN2 cores.

Sharding: data-parallel over the batch dim, 8 graphs per core. Host-side
packing (our chosen input layout, applied to the full inputs):
  * adj rows are pre-scaled by the node mask (row scaling commutes with the
    matmul chain, and masked BN stats need the masked product anyway), then
    transposed so the contraction dim m lands on SBUF partitions.
  * adjT_masked and x are packed into one per-pair-of-graphs "blob" so each
    load is a single large DMA (the toolchain encodes at most ONE semaphore
    wait per instruction, so every matmul must have a single upstream DMA).

Per-core device pipeline:
  chain1 (per graph):  tT[d, n] = sum_m x[m, d] * adjTm[m, n]      (f32r PE)
  chain2 (per graph):  OT[e, n] = sum_d W[d, e] * tT[d, n]         (f32r PE)
  bn_stats/bn_aggr over the free (n) axis of OT in PSUM -> per-core
  (sum, sumsq)[e]; 2 KB AllReduce across the 8 cores; then
  scale[e] = gamma*rsqrt(var+eps), shift[e] = beta - mean*scale,
  W' = W * scale (via a PE outer-product broadcast of scale),
  out[n, e] = relu(mask[n] * (sum_d tT[d,n]*W'[d,e] + shift[e]))   (f32r PE
  + one K=1 matmul adding the shift row + one ACT relu with per-partition
  mask scale), stored via SWDGE so loads (HWDGE) and stores (SWDGE) each
  use every DMA semaphore lane at most once.
"""

import numpy as np

import concourse.bass as bass
import concourse.mybir as mybir
import concourse.tile as tile
from concourse.bass_utils import run_bass_kernel_spmd
from concourse.vector_clock import ScopedClock, VectorClock

B, N, DIN, DOUT = 64, 512, 256, 256
EPS = 1e-5
NCORES = 8
GPC = B // NCORES          # graphs per core
NPAIR = GPC // 2           # paired loads/stores
P = 128
NC_N = N // P              # 4
NC_M = N // P              # 4
NC_D = DIN // P            # 2
NC_E = DOUT // P           # 2

f32 = mybir.dt.float32
f32r = mybir.dt.float32r
bf16 = mybir.dt.bfloat16

# per-graph blob columns: adjT_masked [p, mc, n] then x [p, mc, d]
ADJW = NC_M * N            # 2048
XW = NC_M * DIN            # 1024
GBLOB = ADJW + XW          # 3072 per graph
BLOBW = 2 * GBLOB          # per pair

# aux columns
W0 = 0                         # W packed [p, dc, e] -> 512 cols
IDENT0 = W0 + NC_D * DOUT      # 512
ONES0 = IDENT0 + P             # 640 (128 cols of 1.0; row 0 used as ones-row)
GAMMA0 = ONES0 + P             # 768
BETA0 = GAMMA0 + NC_E          # 770
MASKT0 = BETA0 + NC_E          # 772 (maskT[p, g*4+c] = mask[b, c*128+p])
INVN0 = MASKT0 + GPC * NC_N    # 804
EPS0 = INVN0 + 1               # 805
AUXW = EPS0 + 1                # 806

ActFn = mybir.ActivationFunctionType
Alu = mybir.AluOpType


class _TileContext1W(tile.TileContext):
    """Split the tail drain's multi-waits into single-wait sequencer nops
    (this walrus build encodes at most one sync wait per instruction)."""

    def _drain_and_barrier(self, tick_clock, wait_clock):
        gc = tick_clock.global_clock
        n = len(gc)
        for p in range(n):
            t = gc[p]
            if t > 0:
                single = VectorClock([t if i == p else 0 for i in range(n)])
                nop = self.nc.sync.nop(nofuse=True, hint=f"drain_split_{p}")
                wait_clock.add_sem_waits(nop.ins, ScopedClock({None: single}))
        self.nc.sync.drain()
        self.nc.all_engine_barrier()
        assert self.sems is not None
        popped = self.nc._tile_sem_poison_stack.pop()
        assert popped is self._sem_poison
        self.nc.clear_and_free_semaphores(list(self.sems.allocated().values()))
        self.nc.all_engine_barrier()


def _build_nc():
    nc = bass.Bass(num_devices=NCORES)
    blob_d = nc.dram_tensor("blob", [GPC, P, GBLOB], f32r, kind="ExternalInput")
    aux_d = nc.dram_tensor("aux", [P, AUXW], f32r, kind="ExternalInput")
    out_d = nc.dram_tensor("out", [GPC, N, DOUT], f32, kind="ExternalOutput")

    with _TileContext1W(nc) as tc:
        with (
            tc.tile_pool(name="aux_p", bufs=1) as aux_p,
            tc.tile_pool(name="blob_p", bufs=GPC) as blob_p,
            tc.tile_pool(name="tT_p", bufs=2 * GPC) as tT_p,
            tc.tile_pool(name="row_p", bufs=1) as row_p,
            tc.tile_pool(name="o_p", bufs=NPAIR) as o_p,
            tc.tile_pool(name="st_p", bufs=1) as st_p,
            tc.tile_pool(name="dram", bufs=2, space="DRAM") as dram_p,
        ):
            aux = aux_p.tile([P, AUXW], f32r)
            nc.gpsimd.dma_start(out=aux, in_=aux_d[:, :])
            auxf = aux.bitcast(f32)
            ident_f32 = auxf[:, IDENT0:IDENT0 + P]
            ones_row = aux[0:1, ONES0:ONES0 + P]
            gamma_ap = auxf[:, GAMMA0:GAMMA0 + NC_E]
            beta_ap = auxf[:, BETA0:BETA0 + NC_E]
            invn_ap = auxf[:, INVN0:INVN0 + 1]
            eps_ap = auxf[:, EPS0:EPS0 + 1]

            tT_tiles = []
            osb_tiles = []

            with (
                tc.tile_pool(name="ps_g1", bufs=1, space="PSUM") as ps_g1,
                tc.tile_pool(name="ps_tT", bufs=4, space="PSUM") as ps_tT,
                tc.tile_pool(name="ps_ot", bufs=2, space="PSUM") as ps_ot,
            ):
                # observer gadgets: absorb the aux-DMA wait on PE/ACT/DVE
                g1 = ps_g1.tile([1, 1], f32)
                nc.tensor.matmul(
                    g1[:, :], auxf[0:1, ONES0:ONES0 + 1],
                    auxf[0:1, ONES0:ONES0 + 1], start=True, stop=True,
                )
                gsc = st_p.tile([P, 2], f32, tag="gadget")
                nc.scalar.copy(out=gsc[:, 0:1], in_=eps_ap)
                nc.vector.tensor_copy(out=gsc[:, 1:2], in_=invn_ap)
                # read g1 so its PSUM bank is reader-released before recycling
                gr1 = st_p.tile([1, 1], f32, tag="gadget3")
                nc.vector.tensor_copy(out=gr1, in_=g1[:, :])

                st = st_p.tile([P, NC_E, GPC, 6], f32)

                blobs = []
                for g in range(GPC):
                    blob_g = blob_p.tile([P, GBLOB], f32r, tag="blob", name=f"blob{g}")
                    nc.sync.dma_start(out=blob_g, in_=blob_d[g, :, :])
                    blobs.append(blob_g)
                for g in range(GPC):
                    if True:
                        blob = blobs[g]
                        off = 0
                        # chain1: tT[d, n] = sum_m x[m, d] * adjTm[m, n]
                        # (one PSUM/SBUF tile per dc so the evac of dc0 and
                        #  the first chain2 matmuls overlap chain1 of dc1)
                        tT = []
                        for dc in range(NC_D):
                            tT_ps = ps_tT.tile([P, N], f32, tag="tT",
                                               name=f"tTps{g}_{dc}")
                            for kc in range(NC_M):
                                nc.tensor.matmul(
                                    tT_ps[:, :],
                                    blob[:, off + ADJW + kc * DIN + dc * P:
                                         off + ADJW + kc * DIN + (dc + 1) * P],
                                    blob[:, off + kc * N:off + (kc + 1) * N],
                                    start=(kc == 0), stop=(kc == NC_M - 1),
                                )
                            tT_dc = tT_p.tile([P, N], f32r, tag="tT",
                                              name=f"tT{g}_{dc}")
                            last_evac = nc.scalar.copy(out=tT_dc, in_=tT_ps[:, :])
                            tT.append(tT_dc)
                        tT_tiles.append(tT)

                        # chain2: OT[e, n] = sum_d W[d, e] * tT[d, n]
                        ldw = None
                        if g >= 1:
                            # absorb DVE(bn_stats g-1) before the ot_ps WAR
                            ldw = nc.tensor.ldweights(
                                weights=st[0:1, NC_E - 1, g - 1, 0:1]
                                .bitcast(bf16))
                        for ec in range(NC_E):
                            ot_ps = ps_ot.tile([P, N], f32, tag="ot",
                                               name=f"ot{g}_{ec}")
                            for dc in range(NC_D):
                                mm = nc.tensor.matmul(
                                    ot_ps[:, :],
                                    aux[:, dc * DOUT + ec * P:
                                        dc * DOUT + (ec + 1) * P],
                                    tT[dc][:, :],
                                    start=(dc == 0), stop=(dc == NC_D - 1),
                                )
                                if ldw is not None:
                                    tile.add_dep_helper(
                                        mm.ins, ldw.ins, sync=False,
                                        reason="chain2 after bn-observer ldw")
                                    ldw = None
                            # masked stats straight off PSUM (free axis = n)
                            nc.vector.bn_stats(
                                out=st[:, ec, g, :], in_=ot_ps[:, :])

                # --- stats -> (sum, sumsq) -> AllReduce ---
                mv = st_p.tile([P, NC_E, 2], f32)
                for ec in range(NC_E):
                    nc.vector.bn_aggr(out=mv[:, ec, :], in_=st[:, ec, :, :])
                cnt = float(GPC * N)  # bn count per core (incl. masked zeros)
                pack = st_p.tile([P, 2 * NC_E], f32)
                for ec in range(NC_E):
                    nc.vector.tensor_scalar_mul(
                        out=pack[:, ec:ec + 1], in0=mv[:, ec, 0:1], scalar1=cnt)
                    nc.vector.tensor_scalar(
                        out=pack[:, NC_E + ec:NC_E + ec + 1],
                        in0=mv[:, ec, 0:1],
                        scalar1=mv[:, ec, 0:1], scalar2=mv[:, ec, 1:2],
                        op0=Alu.mult, op1=Alu.add,
                    )
                    nc.vector.tensor_scalar_mul(
                        out=pack[:, NC_E + ec:NC_E + ec + 1],
                        in0=pack[:, NC_E + ec:NC_E + ec + 1], scalar1=cnt)

                ar_in = dram_p.tile([P, 2 * NC_E], f32)
                ar_out = dram_p.tile([P, 2 * NC_E], f32)
                nc.gpsimd.dma_start(out=ar_in[:, :], in_=pack)
                nc.gpsimd.collective_compute(
                    "AllReduce", Alu.add,
                    replica_groups=[list(range(NCORES))],
                    ins=[ar_in[:, :].opt()],
                    outs=[ar_out[:, :].opt()],
                )
                sq = st_p.tile([P, 2 * NC_E], f32)
                sq_dma = nc.gpsimd.dma_start(out=sq, in_=ar_out[:, :])

            with (
                tc.tile_pool(name="ps_w", bufs=3, space="PSUM") as ps_w,
                tc.tile_pool(name="ps_warm", bufs=1, space="PSUM") as ps_warm,
                tc.tile_pool(name="ps2", bufs=4, space="PSUM") as ps2,
            ):
                # PE observers: a psum-free ldweights absorbs the pre-AR
                # DVE tick; three aux-reading matmuls touch the recycled ps_w
                # banks (each needs the pool-transition engine tick exactly
                # once); DVE then reads them so later writers are gated by
                # reader-completion (a single already-covered wait) instead.

                # --- scale/shift (all [128, NC_E], e on partitions) ---
                mean = st_p.tile([P, NC_E], f32)
                var = st_p.tile([P, NC_E], f32)
                m2 = st_p.tile([P, NC_E], f32)
                sd = st_p.tile([P, NC_E], f32)
                rs = st_p.tile([P, NC_E], f32)
                scale = st_p.tile([P, NC_E], f32)
                shift = st_p.tile([P, NC_E], f32)
                nc.vector.tensor_scalar_mul(out=mean, in0=sq[:, 0:NC_E],
                                            scalar1=invn_ap)
                nc.vector.tensor_scalar_mul(out=var, in0=sq[:, NC_E:2 * NC_E],
                                            scalar1=invn_ap)
                nc.vector.tensor_mul(out=m2, in0=mean, in1=mean)
                nc.vector.tensor_sub(out=var, in0=var, in1=m2)
                nc.scalar.activation(out=sd, in_=var, func=ActFn.Sqrt,
                                     bias=eps_ap, scale=1.0)
                nc.vector.reciprocal(out=rs, in_=sd)
                nc.vector.tensor_mul(out=scale, in0=rs, in1=gamma_ap)
                nc.vector.tensor_mul(out=m2, in0=mean, in1=scale)
                nc.vector.tensor_sub(out=shift, in0=beta_ap, in1=m2)
                # rows: scale/shift transposed to [1, DOUT]
                # (psum-free ldweights first: absorb the DVE tick so the
                #  recycled-bank write carries only the PE-engine wait)
                # ACT observer: forced sync dep on the last pre-AR ACT
                # engine op so post-AR ACT PSUM reads carry only their RAW
                actj = st_p.tile([P, 1], f32, tag="actj")
                act_obs = nc.scalar.copy(out=actj, in_=gsc[:, 0:1])
                tile.add_dep_helper(
                    act_obs.ins, last_evac.ins, sync=True,
                    reason="absorb ACT engine tick across psum pool recycle")
                nc.tensor.ldweights(weights=scale[0:1, 0:1].bitcast(bf16))
                rows_ps = ps_w.tile([1, 2, DOUT], f32, tag="w")
                for ec in range(NC_E):
                    nc.tensor.transpose(
                        rows_ps[:, 0, ec * P:(ec + 1) * P],
                        scale[:, ec:ec + 1], ident_f32)
                    nc.tensor.transpose(
                        rows_ps[:, 1, ec * P:(ec + 1) * P],
                        shift[:, ec:ec + 1], ident_f32)
                rows = row_p.tile([1, 2, DOUT], f32r)
                nc.vector.tensor_copy(out=rows[:, 0, :], in_=rows_ps[:, 0, :])
                nc.vector.tensor_copy(out=rows[:, 1, :], in_=rows_ps[:, 1, :])
                # scale broadcast [128, DOUT] via ones-col x scale-row
                nc.tensor.ldweights(weights=rows[0:1, 1, 0:1].bitcast(bf16))
                scbc_ps = ps_w.tile([P, DOUT], f32, tag="w")
                nc.tensor.matmul(scbc_ps[:, :], ones_row, rows[:, 0, :],
                                 start=True, stop=True)
                scbc = row_p.tile([P, DOUT], f32)
                nc.vector.tensor_copy(out=scbc, in_=scbc_ps[:, :])
                wp = row_p.tile([P, NC_D, DOUT], f32r)
                for dc in range(NC_D):
                    nc.vector.tensor_mul(
                        out=wp[:, dc, :],
                        in0=auxf[:, dc * DOUT:(dc + 1) * DOUT], in1=scbc)

                # HAM warm-up: the PE sat idle through the AllReduce and has
                # been clock-gated to 1.2 GHz; ~3.5us of junk matmuls bring it
                # back to 2.4 GHz before the output matmuls. Gated on the AR
                # result so the scheduler cannot hoist them earlier.
                warm_ps = ps_warm.tile([P, N], f32)
                for wi in range(16):
                    wmm = nc.tensor.matmul(
                        warm_ps[:, :], ones_row, aux[0:1, 0:N],
                        start=(wi == 0), stop=(wi == 15),
                    )
                    if wi == 1:
                        # wi==0 absorbs the recycled-bank PE-engine tick; the
                        # AR gate goes on wi==1 (ordered after wi==0 by the
                        # shared accumulation tile)
                        tile.add_dep_helper(
                            wmm.ins, sq_dma.ins, sync=True,
                            reason="PE warm-up runs after the AllReduce")

                for g in range(GPC):
                    if g >= 2:
                        # dummy ldweights: absorb the ACT tick (relu of g-2)
                        # before this graph's PSUM-slot-recycling matmuls
                        nc.tensor.ldweights(
                            weights=osb_tiles[g - 2][0:1, 0, 0, 0:64]
                            .bitcast(bf16))
                    tT = tT_tiles[g]
                    z_tiles = []
                    for jp in range(NC_N // 2):
                        z_ps = ps2.tile([P, 2, DOUT], f32, tag="z",
                                        name=f"z{g}_{jp}")
                        z_tiles.append(z_ps)
                        for jh in range(2):
                            j = 2 * jp + jh
                            nc.tensor.matmul(
                                z_ps[:, jh, :], ones_row, rows[:, 1, :],
                                start=True, stop=False,
                            )
                            for dc in range(NC_D):
                                nc.tensor.matmul(
                                    z_ps[:, jh, :],
                                    tT[dc][:, j * P:(j + 1) * P],
                                    wp[:, dc, :],
                                    start=False, stop=(dc == NC_D - 1),
                                )
                    if g % 2 == 0:
                        osb = o_p.tile([P, 2, NC_N, DOUT], f32, tag="osb")
                    else:
                        osb = osb_tiles[g - 1]
                    osb_tiles.append(osb)
                    for j in range(NC_N):
                        m_ap = auxf[:, MASKT0 + g * NC_N + j:
                                    MASKT0 + g * NC_N + j + 1]
                        z_in = z_tiles[j // 2][:, j % 2, :]
                        if (g // 2) % 2 == 0:
                            nc.scalar.activation(
                                out=osb[:, g % 2, j, :], in_=z_in,
                                func=ActFn.Relu, bias=0.0, scale=m_ap,
                            )
                        else:
                            nc.vector.tensor_scalar(
                                out=osb[:, g % 2, j, :], in0=z_in,
                                scalar1=m_ap, scalar2=0.0,
                                op0=Alu.mult, op1=Alu.max,
                            )
                    if g % 2 == 1:
                        pair = g // 2
                        nc.gpsimd.dma_start(
                            out=out_d[2 * pair:2 * pair + 2]
                                .rearrange("g (c p) e -> p g c e", p=P),
                            in_=osb,
                        )
    return nc


_CACHE = {}


def _get_nc():
    if "nc" not in _CACHE:
        _CACHE["nc"] = _build_nc()
    return _CACHE["nc"]


def kernel(x, adj, mask, weight, bias, gamma, beta):
    x = np.asarray(x, dtype=np.float32)
    adj = np.asarray(adj, dtype=np.float32)
    mask = np.asarray(mask, dtype=np.float32)
    weight = np.asarray(weight, dtype=np.float32)
    gamma = np.asarray(gamma, dtype=np.float32)
    beta = np.asarray(beta, dtype=np.float32)
    # bias cancels exactly in train-mode batchnorm (the mean absorbs it).

    n_tot = float(mask.sum())
    inv_n = np.float32(1.0 / n_tot)

    w_pack = weight.reshape(NC_D, P, DOUT).transpose(1, 0, 2).reshape(P, NC_D * DOUT)
    ident = np.eye(P, dtype=np.float32)
    gam = gamma.reshape(NC_E, P).T.copy()
    bet = beta.reshape(NC_E, P).T.copy()

    in_maps = []
    for c in range(NCORES):
        bs = slice(c * GPC, (c + 1) * GPC)
        adjm = adj[bs] * mask[bs][:, :, None]          # [GPC, n, m]
        adjT = adjm.transpose(0, 2, 1)                 # [GPC, m, n]
        blob_adj = adjT.reshape(GPC, NC_M, P, N).transpose(0, 2, 1, 3) \
                       .reshape(GPC, P, ADJW)
        blob_x = x[bs].reshape(GPC, NC_M, P, DIN).transpose(0, 2, 1, 3) \
                      .reshape(GPC, P, XW)
        blob = np.concatenate([blob_adj, blob_x], axis=2)   # [GPC, P, GBLOB]

        maskT = mask[bs].reshape(GPC, NC_N, P).transpose(2, 0, 1) \
                        .reshape(P, GPC * NC_N)
        aux = np.empty((P, AUXW), dtype=np.float32)
        aux[:, W0:W0 + NC_D * DOUT] = w_pack
        aux[:, IDENT0:IDENT0 + P] = ident
        aux[:, ONES0:ONES0 + P] = 1.0
        aux[:, GAMMA0:GAMMA0 + NC_E] = gam
        aux[:, BETA0:BETA0 + NC_E] = bet
        aux[:, MASKT0:MASKT0 + GPC * NC_N] = maskT
        aux[:, INVN0] = inv_n
        aux[:, EPS0] = np.float32(EPS)
        in_maps.append(dict(blob=np.ascontiguousarray(blob, dtype=np.float32),
                            aux=np.ascontiguousarray(aux)))

    nc = _get_nc()
    res = run_bass_kernel_spmd(nc, in_maps, core_ids=list(range(NCORES)))
    out = np.concatenate([r["out"] for r in res.results], axis=0)
    return out.reshape(B, N, DOUT)


# revision 25
# speedup vs baseline: 33526.0607x; 1.0003x over previous
"""GCN block (adj @ x @ W -> masked BatchNorm(train) -> relu) on 8 TRN2 cores.

Sharding: data-parallel over the batch dim, 8 graphs per core. Host-side
packing (our chosen input layout, applied to the full inputs):
  * adj rows are pre-scaled by the node mask (row scaling commutes with the
    matmul chain, and masked BN stats need the masked product anyway), then
    transposed so the contraction dim m lands on SBUF partitions.
  * adjT_masked and x are packed into one per-pair-of-graphs "blob" so each
    load is a single large DMA (the toolchain encodes at most ONE semaphore
    wait per instruction, so every matmul must have a single upstream DMA).

Per-core device pipeline:
  chain1 (per graph):  tT[d, n] = sum_m x[m, d] * adjTm[m, n]      (f32r PE)
  chain2 (per graph):  OT[e, n] = sum_d W[d, e] * tT[d, n]         (f32r PE)
  bn_stats/bn_aggr over the free (n) axis of OT in PSUM -> per-core
  (sum, sumsq)[e]; 2 KB AllReduce across the 8 cores; then
  scale[e] = gamma*rsqrt(var+eps), shift[e] = beta - mean*scale,
  W' = W * scale (via a PE outer-product broadcast of scale),
  out[n, e] = relu(mask[n] * (sum_d tT[d,n]*W'[d,e] + shift[e]))   (f32r PE
  + one K=1 matmul adding the shift row + one ACT relu with per-partition
  mask scale), stored via SWDGE so loads (HWDGE) and stores (SWDGE) each
  use every DMA semaphore lane at most once.
"""

import numpy as np

import concourse.bass as bass
import concourse.mybir as mybir
import concourse.tile as tile
from concourse.bass_utils import run_bass_kernel_spmd
from concourse.vector_clock import ScopedClock, VectorClock

B, N, DIN, DOUT = 64, 512, 256, 256
EPS = 1e-5
NCORES = 8
GPC = B // NCORES          # graphs per core
NPAIR = GPC // 2           # paired loads/stores
P = 128
NC_N = N // P              # 4
NC_M = N // P              # 4
NC_D = DIN // P            # 2
NC_E = DOUT // P           # 2

f32 = mybir.dt.float32
f32r = mybir.dt.float32r
bf16 = mybir.dt.bfloat16

# per-graph blob columns: adjT_masked [p, mc, n] then x [p, mc, d]
ADJW = NC_M * N            # 2048
XW = NC_M * DIN            # 1024
GBLOB = ADJW + XW          # 3072 per graph
BLOBW = 2 * GBLOB          # per pair

# aux columns
W0 = 0                         # W packed [p, dc, e] -> 512 cols
IDENT0 = W0 + NC_D * DOUT      # 512
ONES0 = IDENT0 + P             # 640 (128 cols of 1.0; row 0 used as ones-row)
GAMMA0 = ONES0 + P             # 768
BETA0 = GAMMA0 + NC_E          # 770
MASKT0 = BETA0 + NC_E          # 772 (maskT[p, g*4+c] = mask[b, c*128+p])
INVN0 = MASKT0 + GPC * NC_N    # 804
EPS0 = INVN0 + 1               # 805
AUXW = EPS0 + 1                # 806

ActFn = mybir.ActivationFunctionType
Alu = mybir.AluOpType


class _TileContext1W(tile.TileContext):
    """Split the tail drain's multi-waits into single-wait sequencer nops
    (this walrus build encodes at most one sync wait per instruction)."""

    def _drain_and_barrier(self, tick_clock, wait_clock):
        gc = tick_clock.global_clock
        n = len(gc)
        for p in range(n):
            t = gc[p]
            if t > 0:
                single = VectorClock([t if i == p else 0 for i in range(n)])
                nop = self.nc.sync.nop(nofuse=True, hint=f"drain_split_{p}")
                wait_clock.add_sem_waits(nop.ins, ScopedClock({None: single}))
        self.nc.sync.drain()
        self.nc.all_engine_barrier()
        assert self.sems is not None
        popped = self.nc._tile_sem_poison_stack.pop()
        assert popped is self._sem_poison
        self.nc.clear_and_free_semaphores(list(self.sems.allocated().values()))
        self.nc.all_engine_barrier()


def _build_nc():
    nc = bass.Bass(num_devices=NCORES)
    blob_d = nc.dram_tensor("blob", [GPC, P, GBLOB], f32r, kind="ExternalInput")
    aux_d = nc.dram_tensor("aux", [P, AUXW], f32r, kind="ExternalInput")
    out_d = nc.dram_tensor("out", [GPC, N, DOUT], f32, kind="ExternalOutput")

    with _TileContext1W(nc) as tc:
        with (
            tc.tile_pool(name="aux_p", bufs=1) as aux_p,
            tc.tile_pool(name="blob_p", bufs=GPC) as blob_p,
            tc.tile_pool(name="tT_p", bufs=2 * GPC) as tT_p,
            tc.tile_pool(name="row_p", bufs=1) as row_p,
            tc.tile_pool(name="o_p", bufs=NPAIR) as o_p,
            tc.tile_pool(name="st_p", bufs=1) as st_p,
            tc.tile_pool(name="dram", bufs=2, space="DRAM") as dram_p,
        ):
            aux = aux_p.tile([P, AUXW], f32r)
            nc.gpsimd.dma_start(out=aux, in_=aux_d[:, :])
            auxf = aux.bitcast(f32)
            ident_f32 = auxf[:, IDENT0:IDENT0 + P]
            ones_row = aux[0:1, ONES0:ONES0 + P]
            gamma_ap = auxf[:, GAMMA0:GAMMA0 + NC_E]
            beta_ap = auxf[:, BETA0:BETA0 + NC_E]
            invn_ap = auxf[:, INVN0:INVN0 + 1]
            eps_ap = auxf[:, EPS0:EPS0 + 1]

            tT_tiles = []
            osb_tiles = []

            with (
                tc.tile_pool(name="ps_g1", bufs=1, space="PSUM") as ps_g1,
                tc.tile_pool(name="ps_tT", bufs=4, space="PSUM") as ps_tT,
                tc.tile_pool(name="ps_ot", bufs=2, space="PSUM") as ps_ot,
            ):
                # observer gadgets: absorb the aux-DMA wait on PE/ACT/DVE
                g1 = ps_g1.tile([1, 1], f32)
                nc.tensor.matmul(
                    g1[:, :], auxf[0:1, ONES0:ONES0 + 1],
                    auxf[0:1, ONES0:ONES0 + 1], start=True, stop=True,
                )
                gsc = st_p.tile([P, 2], f32, tag="gadget")
                nc.scalar.copy(out=gsc[:, 0:1], in_=eps_ap)
                nc.vector.tensor_copy(out=gsc[:, 1:2], in_=invn_ap)
                # read g1 so its PSUM bank is reader-released before recycling
                gr1 = st_p.tile([1, 1], f32, tag="gadget3")
                nc.vector.tensor_copy(out=gr1, in_=g1[:, :])

                st = st_p.tile([P, NC_E, GPC, 6], f32)

                blobs = []
                for g in range(GPC):
                    blob_g = blob_p.tile([P, GBLOB], f32r, tag="blob", name=f"blob{g}")
                    nc.sync.dma_start(out=blob_g, in_=blob_d[g, :, :])
                    blobs.append(blob_g)
                for g in range(GPC):
                    if True:
                        blob = blobs[g]
                        off = 0
                        # chain1: tT[d, n] = sum_m x[m, d] * adjTm[m, n]
                        # (one PSUM/SBUF tile per dc so the evac of dc0 and
                        #  the first chain2 matmuls overlap chain1 of dc1)
                        tT = []
                        for dc in range(NC_D):
                            tT_ps = ps_tT.tile([P, N], f32, tag="tT",
                                               name=f"tTps{g}_{dc}")
                            for kc in range(NC_M):
                                nc.tensor.matmul(
                                    tT_ps[:, :],
                                    blob[:, off + ADJW + kc * DIN + dc * P:
                                         off + ADJW + kc * DIN + (dc + 1) * P],
                                    blob[:, off + kc * N:off + (kc + 1) * N],
                                    start=(kc == 0), stop=(kc == NC_M - 1),
                                )
                            tT_dc = tT_p.tile([P, N], f32r, tag="tT",
                                              name=f"tT{g}_{dc}")
                            last_evac = nc.scalar.copy(out=tT_dc, in_=tT_ps[:, :])
                            tT.append(tT_dc)
                        tT_tiles.append(tT)

                        # chain2: OT[e, n] = sum_d W[d, e] * tT[d, n]
                        ldw = None
                        if g >= 1:
                            # absorb DVE(bn_stats g-1) before the ot_ps WAR
                            ldw = nc.tensor.ldweights(
                                weights=st[0:1, NC_E - 1, g - 1, 0:1]
                                .bitcast(bf16))
                        for ec in range(NC_E):
                            ot_ps = ps_ot.tile([P, N], f32, tag="ot",
                                               name=f"ot{g}_{ec}")
                            for dc in range(NC_D):
                                mm = nc.tensor.matmul(
                                    ot_ps[:, :],
                                    aux[:, dc * DOUT + ec * P:
                                        dc * DOUT + (ec + 1) * P],
                                    tT[dc][:, :],
                                    start=(dc == 0), stop=(dc == NC_D - 1),
                                )
                                if ldw is not None:
                                    tile.add_dep_helper(
                                        mm.ins, ldw.ins, sync=False,
                                        reason="chain2 after bn-observer ldw")
                                    ldw = None
                            # masked stats straight off PSUM (free axis = n)
                            last_bn = nc.vector.bn_stats(
                                out=st[:, ec, g, :], in_=ot_ps[:, :])

                # --- stats -> (sum, sumsq) -> AllReduce ---
                mv = st_p.tile([P, NC_E, 2], f32)
                for ec in range(NC_E):
                    nc.vector.bn_aggr(out=mv[:, ec, :], in_=st[:, ec, :, :])
                cnt = float(GPC * N)  # bn count per core (incl. masked zeros)
                pack = st_p.tile([P, 2 * NC_E], f32)
                for ec in range(NC_E):
                    nc.vector.tensor_scalar_mul(
                        out=pack[:, ec:ec + 1], in0=mv[:, ec, 0:1], scalar1=cnt)
                    nc.vector.tensor_scalar(
                        out=pack[:, NC_E + ec:NC_E + ec + 1],
                        in0=mv[:, ec, 0:1],
                        scalar1=mv[:, ec, 0:1], scalar2=mv[:, ec, 1:2],
                        op0=Alu.mult, op1=Alu.add,
                    )
                    nc.vector.tensor_scalar_mul(
                        out=pack[:, NC_E + ec:NC_E + ec + 1],
                        in0=pack[:, NC_E + ec:NC_E + ec + 1], scalar1=cnt)

                ar_in = dram_p.tile([P, 2 * NC_E], f32)
                ar_out = dram_p.tile([P, 2 * NC_E], f32)
                nc.gpsimd.dma_start(out=ar_in[:, :], in_=pack)
                nc.gpsimd.collective_compute(
                    "AllReduce", Alu.add,
                    replica_groups=[list(range(NCORES))],
                    ins=[ar_in[:, :].opt()],
                    outs=[ar_out[:, :].opt()],
                )
                sq = st_p.tile([P, 2 * NC_E], f32)
                sq_dma = nc.gpsimd.dma_start(out=sq, in_=ar_out[:, :])

            with (
                tc.tile_pool(name="ps_w", bufs=3, space="PSUM") as ps_w,
                tc.tile_pool(name="ps_warm", bufs=1, space="PSUM") as ps_warm,
                tc.tile_pool(name="ps2", bufs=4, space="PSUM") as ps2,
            ):
                # PE observers: a psum-free ldweights absorbs the pre-AR
                # DVE tick; three aux-reading matmuls touch the recycled ps_w
                # banks (each needs the pool-transition engine tick exactly
                # once); DVE then reads them so later writers are gated by
                # reader-completion (a single already-covered wait) instead.

                # --- scale/shift (all [128, NC_E], e on partitions) ---
                mean = st_p.tile([P, NC_E], f32)
                var = st_p.tile([P, NC_E], f32)
                m2 = st_p.tile([P, NC_E], f32)
                sd = st_p.tile([P, NC_E], f32)
                rs = st_p.tile([P, NC_E], f32)
                scale = st_p.tile([P, NC_E], f32)
                shift = st_p.tile([P, NC_E], f32)
                nc.vector.tensor_scalar_mul(out=mean, in0=sq[:, 0:NC_E],
                                            scalar1=invn_ap)
                nc.vector.tensor_scalar_mul(out=var, in0=sq[:, NC_E:2 * NC_E],
                                            scalar1=invn_ap)
                nc.vector.tensor_mul(out=m2, in0=mean, in1=mean)
                nc.vector.tensor_sub(out=var, in0=var, in1=m2)
                nc.scalar.activation(out=sd, in_=var, func=ActFn.Sqrt,
                                     bias=eps_ap, scale=1.0)
                nc.vector.reciprocal(out=rs, in_=sd)
                nc.vector.tensor_mul(out=scale, in0=rs, in1=gamma_ap)
                nc.vector.tensor_mul(out=m2, in0=mean, in1=scale)
                nc.vector.tensor_sub(out=shift, in0=beta_ap, in1=m2)
                # rows: scale/shift transposed to [1, DOUT]
                # (psum-free ldweights first: absorb the DVE tick so the
                #  recycled-bank write carries only the PE-engine wait)
                # ACT observer: forced sync dep on the last pre-AR ACT
                # engine op so post-AR ACT PSUM reads carry only their RAW
                actj = st_p.tile([P, 1], f32, tag="actj")
                act_obs = nc.scalar.copy(out=actj, in_=gsc[:, 0:1])
                tile.add_dep_helper(
                    act_obs.ins, last_evac.ins, sync=True,
                    reason="absorb ACT engine tick across psum pool recycle")
                dvej = st_p.tile([P, 1], f32, tag="dvej")
                dve_obs = nc.vector.tensor_copy(out=dvej, in_=gsc[:, 0:1])
                tile.add_dep_helper(
                    dve_obs.ins, last_bn.ins, sync=True,
                    reason="absorb DVE engine tick across psum pool recycle")
                nc.tensor.ldweights(weights=scale[0:1, 0:1].bitcast(bf16))
                rows_ps = ps_w.tile([1, 2, DOUT], f32, tag="w")
                for ec in range(NC_E):
                    nc.tensor.transpose(
                        rows_ps[:, 0, ec * P:(ec + 1) * P],
                        scale[:, ec:ec + 1], ident_f32)
                    nc.tensor.transpose(
                        rows_ps[:, 1, ec * P:(ec + 1) * P],
                        shift[:, ec:ec + 1], ident_f32)
                rows = row_p.tile([1, 2, DOUT], f32r)
                nc.vector.tensor_copy(out=rows[:, 0, :], in_=rows_ps[:, 0, :])
                nc.vector.tensor_copy(out=rows[:, 1, :], in_=rows_ps[:, 1, :])
                # scale broadcast [128, DOUT] via ones-col x scale-row
                nc.tensor.ldweights(weights=rows[0:1, 1, 0:1].bitcast(bf16))
                scbc_ps = ps_w.tile([P, DOUT], f32, tag="w")
                nc.tensor.matmul(scbc_ps[:, :], ones_row, rows[:, 0, :],
                                 start=True, stop=True)
                wp = row_p.tile([P, NC_D, DOUT], f32r)
                for dc in range(NC_D):
                    nc.vector.tensor_mul(
                        out=wp[:, dc, :],
                        in0=auxf[:, dc * DOUT:(dc + 1) * DOUT], in1=scbc_ps[:, :])

                # HAM warm-up: the PE sat idle through the AllReduce and has
                # been clock-gated to 1.2 GHz; ~3.5us of junk matmuls bring it
                # back to 2.4 GHz before the output matmuls. Gated on the AR
                # result so the scheduler cannot hoist them earlier.
                warm_ps = ps_warm.tile([P, N], f32)
                for wi in range(16):
                    wmm = nc.tensor.matmul(
                        warm_ps[:, :], ones_row, aux[0:1, 0:N],
                        start=(wi == 0), stop=(wi == 15),
                    )
                    if wi == 1:
                        # wi==0 absorbs the recycled-bank PE-engine tick; the
                        # AR gate goes on wi==1 (ordered after wi==0 by the
                        # shared accumulation tile)
                        tile.add_dep_helper(
                            wmm.ins, sq_dma.ins, sync=True,
                            reason="PE warm-up runs after the AllReduce")

                for g in range(GPC):
                    pair = g // 2
                    half = g % 2
                    if g >= 2:
                        # dummy ldweights: absorb the relu(g-2) engine tick
                        # before this graph's PSUM-slot-recycling matmuls
                        nc.tensor.ldweights(
                            weights=osb_tiles[(g - 2) // 2][0:1, (g - 2) % 2,
                                                            0, 0:64]
                            .bitcast(bf16))
                    tT = tT_tiles[g]
                    z_tiles = []
                    for jp in range(NC_N // 2):
                        z_ps = ps2.tile([P, 2, DOUT], f32, tag="z",
                                        name=f"z{g}_{jp}")
                        z_tiles.append(z_ps)
                        for jh in range(2):
                            j = 2 * jp + jh
                            nc.tensor.matmul(
                                z_ps[:, jh, :], ones_row, rows[:, 1, :],
                                start=True, stop=False,
                            )
                            for dc in range(NC_D):
                                nc.tensor.matmul(
                                    z_ps[:, jh, :],
                                    tT[dc][:, j * P:(j + 1) * P],
                                    wp[:, dc, :],
                                    start=False, stop=(dc == NC_D - 1),
                                )
                    if len(osb_tiles) <= pair:
                        osb = o_p.tile([P, 2, NC_N, DOUT], f32, tag="osb")
                        osb_tiles.append(osb)
                    else:
                        osb = osb_tiles[pair]
                    for j in range(NC_N):
                        m_ap = auxf[:, MASKT0 + g * NC_N + j:
                                    MASKT0 + g * NC_N + j + 1]
                        z_in = z_tiles[j // 2][:, j % 2, :]
                        if (g // 2) % 2 == 0:
                            nc.scalar.activation(
                                out=osb[:, half, j, :], in_=z_in,
                                func=ActFn.Relu, bias=0.0, scale=m_ap,
                            )
                        else:
                            nc.vector.tensor_scalar(
                                out=osb[:, half, j, :], in0=z_in,
                                scalar1=m_ap, scalar2=0.0,
                                op0=Alu.mult, op1=Alu.max,
                            )
                    if half == 1:
                        nc.gpsimd.dma_start(
                            out=out_d[2 * pair:2 * pair + 2]
                                .rearrange("g (c p) e -> p g c e", p=P),
                            in_=osb,
                        )
    return nc


_CACHE = {}


def _get_nc():
    if "nc" not in _CACHE:
        _CACHE["nc"] = _build_nc()
    return _CACHE["nc"]


def kernel(x, adj, mask, weight, bias, gamma, beta):
    x = np.asarray(x, dtype=np.float32)
    adj = np.asarray(adj, dtype=np.float32)
    mask = np.asarray(mask, dtype=np.float32)
    weight = np.asarray(weight, dtype=np.float32)
    gamma = np.asarray(gamma, dtype=np.float32)
    beta = np.asarray(beta, dtype=np.float32)
    # bias cancels exactly in train-mode batchnorm (the mean absorbs it).

    n_tot = float(mask.sum())
    inv_n = np.float32(1.0 / n_tot)

    w_pack = weight.reshape(NC_D, P, DOUT).transpose(1, 0, 2).reshape(P, NC_D * DOUT)
    ident = np.eye(P, dtype=np.float32)
    gam = gamma.reshape(NC_E, P).T.copy()
    bet = beta.reshape(NC_E, P).T.copy()

    in_maps = []
    for c in range(NCORES):
        bs = slice(c * GPC, (c + 1) * GPC)
        adjm = adj[bs] * mask[bs][:, :, None]          # [GPC, n, m]
        adjT = adjm.transpose(0, 2, 1)                 # [GPC, m, n]
        blob_adj = adjT.reshape(GPC, NC_M, P, N).transpose(0, 2, 1, 3) \
                       .reshape(GPC, P, ADJW)
        blob_x = x[bs].reshape(GPC, NC_M, P, DIN).transpose(0, 2, 1, 3) \
                      .reshape(GPC, P, XW)
        blob = np.concatenate([blob_adj, blob_x], axis=2)   # [GPC, P, GBLOB]

        maskT = mask[bs].reshape(GPC, NC_N, P).transpose(2, 0, 1) \
                        .reshape(P, GPC * NC_N)
        aux = np.empty((P, AUXW), dtype=np.float32)
        aux[:, W0:W0 + NC_D * DOUT] = w_pack
        aux[:, IDENT0:IDENT0 + P] = ident
        aux[:, ONES0:ONES0 + P] = 1.0
        aux[:, GAMMA0:GAMMA0 + NC_E] = gam
        aux[:, BETA0:BETA0 + NC_E] = bet
        aux[:, MASKT0:MASKT0 + GPC * NC_N] = maskT
        aux[:, INVN0] = inv_n
        aux[:, EPS0] = np.float32(EPS)
        in_maps.append(dict(blob=np.ascontiguousarray(blob, dtype=np.float32),
                            aux=np.ascontiguousarray(aux)))

    nc = _get_nc()
    res = run_bass_kernel_spmd(nc, in_maps, core_ids=list(range(NCORES)))
    out = np.concatenate([r["out"] for r in res.results], axis=0)
    return out.reshape(B, N, DOUT)


# revision 31
# speedup vs baseline: 33969.3299x; 1.0132x over previous
"""GCN block (adj @ x @ W -> masked BatchNorm(train) -> relu) on 8 TRN2 cores.

Sharding: data-parallel over the batch dim, 8 graphs per core. Host-side
packing (our chosen input layout, applied to the full inputs):
  * adj rows are pre-scaled by the node mask (row scaling commutes with the
    matmul chain, and masked BN stats need the masked product anyway), then
    transposed so the contraction dim m lands on SBUF partitions.
  * adjT_masked and x are packed kc-major into one per-graph "blob" so each
    graph is a single large DMA (the toolchain encodes at most ONE semaphore
    wait per instruction, so every matmul must have a single upstream DMA);
    the last graph's blob is split in two so half of its chain1 overlaps the
    final load, shortening the pre-AllReduce drain.

Per-core device pipeline:
  chain1 (per graph):  tT[d, n] = sum_m x[m, d] * adjTm[m, n]      (f32r PE)
  chain2 (per graph):  OT[e, n] = sum_d W[d, e] * tT[d, n]         (f32r PE)
  bn_stats/bn_aggr over the free (n) axis of OT in PSUM -> per-core
  (sum, sumsq)[e]; 2 KB AllReduce across the 8 cores; then
  scale[e] = gamma*rsqrt(var+eps), shift[e] = beta - mean*scale,
  W' = W * scale (via a PE outer-product broadcast of scale),
  out[n, e] = relu(mask[n] * (sum_d tT[d,n]*W'[d,e] + shift[e]))   (f32r PE
  + one K=1 matmul adding the shift row + one ACT relu with per-partition
  mask scale), stored via SWDGE so loads (HWDGE) and stores (SWDGE) each
  use every DMA semaphore lane at most once.
"""

import numpy as np

import concourse.bass as bass
import concourse.mybir as mybir
import concourse.tile as tile
from concourse.bass_utils import run_bass_kernel_spmd
from concourse.vector_clock import ScopedClock, VectorClock

B, N, DIN, DOUT = 64, 512, 256, 256
EPS = 1e-5
NCORES = 8
GPC = B // NCORES          # graphs per core
NPAIR = GPC // 2           # paired loads/stores
P = 128
NC_N = N // P              # 4
NC_M = N // P              # 4
NC_D = DIN // P            # 2
NC_E = DOUT // P           # 2

f32 = mybir.dt.float32
f32r = mybir.dt.float32r
bf16 = mybir.dt.bfloat16

# per-graph blob columns: adjT_masked [p, mc, n] then x [p, mc, d]
ADJW = NC_M * N            # 2048
XW = NC_M * DIN            # 1024
GBLOB = ADJW + XW          # 3072 per graph
BLOBW = 2 * GBLOB          # per pair

# aux columns
W0 = 0                         # W packed [p, dc, e] -> 512 cols
IDENT0 = W0 + NC_D * DOUT      # 512
ONES0 = IDENT0 + P             # 640 (128 cols of 1.0; row 0 used as ones-row)
GAMMA0 = ONES0 + P             # 768
BETA0 = GAMMA0 + NC_E          # 770
MASKT0 = BETA0 + NC_E          # 772 (maskT[p, g*4+c] = mask[b, c*128+p])
INVN0 = MASKT0 + GPC * NC_N    # 804
EPS0 = INVN0 + 1               # 805
AUXW = EPS0 + 1                # 806

KCB = 512 + 256  # kc-major blob block: [adjT_kc(512) | x_kc(256)]

ActFn = mybir.ActivationFunctionType
Alu = mybir.AluOpType


class _TileContext1W(tile.TileContext):
    """Split the tail drain's multi-waits into single-wait sequencer nops
    (this walrus build encodes at most one sync wait per instruction)."""

    def _drain_and_barrier(self, tick_clock, wait_clock):
        gc = tick_clock.global_clock
        n = len(gc)
        for p in range(n):
            t = gc[p]
            if t > 0:
                single = VectorClock([t if i == p else 0 for i in range(n)])
                nop = self.nc.sync.nop(nofuse=True, hint=f"drain_split_{p}")
                wait_clock.add_sem_waits(nop.ins, ScopedClock({None: single}))
        self.nc.sync.drain()
        self.nc.all_engine_barrier()
        assert self.sems is not None
        popped = self.nc._tile_sem_poison_stack.pop()
        assert popped is self._sem_poison
        self.nc.clear_and_free_semaphores(list(self.sems.allocated().values()))
        self.nc.all_engine_barrier()


def _build_nc():
    nc = bass.Bass(num_devices=NCORES)
    blob_d = nc.dram_tensor("blob", [GPC, P, GBLOB], f32r, kind="ExternalInput")
    aux_d = nc.dram_tensor("aux", [P, AUXW], f32r, kind="ExternalInput")
    out_d = nc.dram_tensor("out", [GPC, N, DOUT], f32, kind="ExternalOutput")

    with _TileContext1W(nc) as tc:
        with (
            tc.tile_pool(name="aux_p", bufs=1) as aux_p,
            tc.tile_pool(name="blob_p", bufs=GPC + 2) as blob_p,
            tc.tile_pool(name="tT_p", bufs=2 * GPC) as tT_p,
            tc.tile_pool(name="row_p", bufs=1) as row_p,
            tc.tile_pool(name="o_p", bufs=NPAIR) as o_p,
            tc.tile_pool(name="st_p", bufs=1) as st_p,
            tc.tile_pool(name="dram", bufs=2, space="DRAM") as dram_p,
        ):
            aux = aux_p.tile([P, AUXW], f32r)
            nc.gpsimd.dma_start(out=aux, in_=aux_d[:, :])
            auxf = aux.bitcast(f32)
            ident_f32 = auxf[:, IDENT0:IDENT0 + P]
            ones_row = aux[0:1, ONES0:ONES0 + P]
            gamma_ap = auxf[:, GAMMA0:GAMMA0 + NC_E]
            beta_ap = auxf[:, BETA0:BETA0 + NC_E]
            invn_ap = auxf[:, INVN0:INVN0 + 1]
            eps_ap = auxf[:, EPS0:EPS0 + 1]

            tT_tiles = []
            osb_tiles = []

            with (
                tc.tile_pool(name="ps_g1", bufs=1, space="PSUM") as ps_g1,
                tc.tile_pool(name="ps_tT", bufs=4, space="PSUM") as ps_tT,
                tc.tile_pool(name="ps_ot", bufs=2, space="PSUM") as ps_ot,
            ):
                # observer gadgets: absorb the aux-DMA wait on PE/ACT/DVE
                g1 = ps_g1.tile([1, 1], f32)
                nc.tensor.matmul(
                    g1[:, :], auxf[0:1, ONES0:ONES0 + 1],
                    auxf[0:1, ONES0:ONES0 + 1], start=True, stop=True,
                )
                gsc = st_p.tile([P, 2], f32, tag="gadget")
                nc.scalar.copy(out=gsc[:, 0:1], in_=eps_ap)
                nc.vector.tensor_copy(out=gsc[:, 1:2], in_=invn_ap)
                # read g1 so its PSUM bank is reader-released before recycling
                gr1 = st_p.tile([1, 1], f32, tag="gadget3")
                nc.vector.tensor_copy(out=gr1, in_=g1[:, :])

                st = st_p.tile([P, NC_E, GPC, 6], f32)

                blobs = []
                for g in range(GPC - 2):
                    blob_g = blob_p.tile([P, GBLOB], f32r, tag="blob", name=f"blob{g}")
                    nc.sync.dma_start(out=blob_g, in_=blob_d[g, :, :])
                    blobs.append(blob_g)
                # last two graphs: half-loads so half of each chain1 overlaps
                # the remaining DMAs (loads 9/10 reuse lanes; their lane-reuse
                # wait is their only dep, so the 1-wait limit holds)
                for g in (GPC - 2, GPC - 1):
                    ha = blob_p.tile([P, GBLOB // 2], f32r, tag="blob",
                                     name=f"b{g}a")
                    hb = blob_p.tile([P, GBLOB // 2], f32r, tag="blob",
                                     name=f"b{g}b")
                    nc.sync.dma_start(out=ha, in_=blob_d[g, :, 0:GBLOB // 2])
                    nc.sync.dma_start(out=hb, in_=blob_d[g, :, GBLOB // 2:GBLOB])
                    blobs.append((ha, hb))
                for g in range(GPC):
                    if True:
                        blob = blobs[g]
                        off = 0
                        # chain1: tT[d, n] = sum_m x[m, d] * adjTm[m, n]
                        # (one PSUM/SBUF tile per dc so the evac of dc0 and
                        #  the first chain2 matmuls overlap chain1 of dc1)
                        tT = []
                        for dc in range(NC_D):
                            tT_ps = ps_tT.tile([P, N], f32, tag="tT",
                                               name=f"tTps{g}_{dc}")
                            for kc in range(NC_M):
                                if isinstance(blob, tuple):
                                    bt = blob[kc // 2]
                                    base = (kc % 2) * KCB
                                else:
                                    bt = blob
                                    base = kc * KCB
                                nc.tensor.matmul(
                                    tT_ps[:, :],
                                    bt[:, base + 512 + dc * P:
                                       base + 512 + (dc + 1) * P],
                                    bt[:, base:base + 512],
                                    start=(kc == 0), stop=(kc == NC_M - 1),
                                )
                            tT_dc = tT_p.tile([P, N], f32r, tag="tT",
                                              name=f"tT{g}_{dc}")
                            last_evac = nc.scalar.copy(out=tT_dc, in_=tT_ps[:, :])
                            tT.append(tT_dc)
                        tT_tiles.append(tT)

                        # chain2: OT[e, n] = sum_d W[d, e] * tT[d, n]
                        ldw = None
                        if g >= 1:
                            # absorb DVE(bn_stats g-1) before the ot_ps WAR
                            ldw = nc.tensor.ldweights(
                                weights=st[0:1, NC_E - 1, g - 1, 0:1]
                                .bitcast(bf16))
                        for ec in range(NC_E):
                            ot_ps = ps_ot.tile([P, N], f32, tag="ot",
                                               name=f"ot{g}_{ec}")
                            for dc in range(NC_D):
                                mm = nc.tensor.matmul(
                                    ot_ps[:, :],
                                    aux[:, dc * DOUT + ec * P:
                                        dc * DOUT + (ec + 1) * P],
                                    tT[dc][:, :],
                                    start=(dc == 0), stop=(dc == NC_D - 1),
                                )
                                if ldw is not None:
                                    tile.add_dep_helper(
                                        mm.ins, ldw.ins, sync=False,
                                        reason="chain2 after bn-observer ldw")
                                    ldw = None
                            # masked stats straight off PSUM (free axis = n)
                            last_bn = nc.vector.bn_stats(
                                out=st[:, ec, g, :], in_=ot_ps[:, :])

                # --- stats -> (sum, sumsq) -> AllReduce ---
                mv = st_p.tile([P, NC_E, 2], f32)
                for ec in range(NC_E):
                    nc.vector.bn_aggr(out=mv[:, ec, :], in_=st[:, ec, :, :])
                cnt = float(GPC * N)  # bn count per core (incl. masked zeros)
                pack = st_p.tile([P, 2 * NC_E], f32)
                for ec in range(NC_E):
                    nc.vector.tensor_scalar_mul(
                        out=pack[:, ec:ec + 1], in0=mv[:, ec, 0:1], scalar1=cnt)
                    nc.vector.tensor_scalar(
                        out=pack[:, NC_E + ec:NC_E + ec + 1],
                        in0=mv[:, ec, 0:1],
                        scalar1=mv[:, ec, 0:1], scalar2=mv[:, ec, 1:2],
                        op0=Alu.mult, op1=Alu.add,
                    )
                    nc.vector.tensor_scalar_mul(
                        out=pack[:, NC_E + ec:NC_E + ec + 1],
                        in0=pack[:, NC_E + ec:NC_E + ec + 1], scalar1=cnt)

                ar_in = dram_p.tile([P, 2 * NC_E], f32)
                ar_out = dram_p.tile([P, 2 * NC_E], f32)
                nc.gpsimd.dma_start(out=ar_in[:, :], in_=pack)
                nc.gpsimd.collective_compute(
                    "AllReduce", Alu.add,
                    replica_groups=[list(range(NCORES))],
                    ins=[ar_in[:, :].opt()],
                    outs=[ar_out[:, :].opt()],
                )
                sq = st_p.tile([P, 2 * NC_E], f32)
                sq_dma = nc.gpsimd.dma_start(out=sq, in_=ar_out[:, :])

            with (
                tc.tile_pool(name="ps_w", bufs=3, space="PSUM") as ps_w,
                tc.tile_pool(name="ps_warm", bufs=1, space="PSUM") as ps_warm,
                tc.tile_pool(name="ps2", bufs=4, space="PSUM") as ps2,
            ):
                # PE observers: a psum-free ldweights absorbs the pre-AR
                # DVE tick; three aux-reading matmuls touch the recycled ps_w
                # banks (each needs the pool-transition engine tick exactly
                # once); DVE then reads them so later writers are gated by
                # reader-completion (a single already-covered wait) instead.

                # --- scale/shift (all [128, NC_E], e on partitions) ---
                var = st_p.tile([P, NC_E], f32)
                m2 = st_p.tile([P, NC_E], f32)
                sd = st_p.tile([P, NC_E], f32)
                rs = st_p.tile([P, NC_E], f32)
                scale = st_p.tile([P, NC_E], f32)
                shift = st_p.tile([P, NC_E], f32)
                mq = st_p.tile([P, 2 * NC_E], f32)
                nc.vector.tensor_scalar_mul(out=mq, in0=sq, scalar1=invn_ap)
                mean = mq[:, 0:NC_E]
                nc.vector.tensor_mul(out=m2, in0=mean, in1=mean)
                nc.vector.tensor_sub(out=var, in0=mq[:, NC_E:2 * NC_E], in1=m2)
                nc.scalar.activation(out=sd, in_=var, func=ActFn.Sqrt,
                                     bias=eps_ap, scale=1.0)
                nc.vector.reciprocal(out=rs, in_=sd)
                nc.vector.tensor_mul(out=scale, in0=rs, in1=gamma_ap)
                nc.vector.tensor_mul(out=m2, in0=mean, in1=scale)
                nc.vector.tensor_sub(out=shift, in0=beta_ap, in1=m2)
                # rows: scale/shift transposed to [1, DOUT]
                # (psum-free ldweights first: absorb the DVE tick so the
                #  recycled-bank write carries only the PE-engine wait)
                # ACT observer: forced sync dep on the last pre-AR ACT
                # engine op so post-AR ACT PSUM reads carry only their RAW
                actj = st_p.tile([P, 1], f32, tag="actj")
                act_obs = nc.scalar.copy(out=actj, in_=gsc[:, 0:1])
                tile.add_dep_helper(
                    act_obs.ins, last_evac.ins, sync=True,
                    reason="absorb ACT engine tick across psum pool recycle")
                dvej = st_p.tile([P, 1], f32, tag="dvej")
                dve_obs = nc.vector.tensor_copy(out=dvej, in_=gsc[:, 0:1])
                tile.add_dep_helper(
                    dve_obs.ins, last_bn.ins, sync=True,
                    reason="absorb DVE engine tick across psum pool recycle")
                nc.tensor.ldweights(weights=scale[0:1, 0:1].bitcast(bf16))
                rows_ps = ps_w.tile([1, 2, DOUT], f32, tag="w")
                for ec in range(NC_E):
                    nc.tensor.transpose(
                        rows_ps[:, 0, ec * P:(ec + 1) * P],
                        scale[:, ec:ec + 1], ident_f32)
                    nc.tensor.transpose(
                        rows_ps[:, 1, ec * P:(ec + 1) * P],
                        shift[:, ec:ec + 1], ident_f32)
                rows = row_p.tile([1, 2, DOUT], f32r)
                nc.vector.tensor_copy(out=rows[:, 0, :], in_=rows_ps[:, 0, :])
                nc.vector.tensor_copy(out=rows[:, 1, :], in_=rows_ps[:, 1, :])
                # scale broadcast [128, DOUT] via ones-col x scale-row
                nc.tensor.ldweights(weights=rows[0:1, 1, 0:1].bitcast(bf16))
                scbc_ps = ps_w.tile([P, DOUT], f32, tag="w")
                nc.tensor.matmul(scbc_ps[:, :], ones_row, rows[:, 0, :],
                                 start=True, stop=True)
                wp = row_p.tile([P, NC_D, DOUT], f32r)
                for dc in range(NC_D):
                    nc.vector.tensor_mul(
                        out=wp[:, dc, :],
                        in0=auxf[:, dc * DOUT:(dc + 1) * DOUT], in1=scbc_ps[:, :])

                # HAM warm-up: the PE sat idle through the AllReduce and has
                # been clock-gated to 1.2 GHz; ~3.5us of junk matmuls bring it
                # back to 2.4 GHz before the output matmuls. Gated on the AR
                # result so the scheduler cannot hoist them earlier.
                warm_ps = ps_warm.tile([P, N], f32)
                for wi in range(16):
                    wmm = nc.tensor.matmul(
                        warm_ps[:, :], ones_row, aux[0:1, 0:N],
                        start=(wi == 0), stop=(wi == 15),
                    )
                    if wi == 1:
                        # wi==0 absorbs the recycled-bank PE-engine tick; the
                        # AR gate goes on wi==1 (ordered after wi==0 by the
                        # shared accumulation tile)
                        tile.add_dep_helper(
                            wmm.ins, sq_dma.ins, sync=True,
                            reason="PE warm-up runs after the AllReduce")

                for g in range(GPC):
                    pair = g // 2
                    half = g % 2
                    if g >= 2:
                        # dummy ldweights: absorb the relu(g-2) engine tick
                        # before this graph's PSUM-slot-recycling matmuls
                        nc.tensor.ldweights(
                            weights=osb_tiles[(g - 2) // 2][0:1, (g - 2) % 2,
                                                            0, 0:64]
                            .bitcast(bf16))
                    tT = tT_tiles[g]
                    z_tiles = []
                    for jp in range(NC_N // 2):
                        z_ps = ps2.tile([P, 2, DOUT], f32, tag="z",
                                        name=f"z{g}_{jp}")
                        z_tiles.append(z_ps)
                        for jh in range(2):
                            j = 2 * jp + jh
                            nc.tensor.matmul(
                                z_ps[:, jh, :], ones_row, rows[:, 1, :],
                                start=True, stop=False,
                            )
                            for dc in range(NC_D):
                                nc.tensor.matmul(
                                    z_ps[:, jh, :],
                                    tT[dc][:, j * P:(j + 1) * P],
                                    wp[:, dc, :],
                                    start=False, stop=(dc == NC_D - 1),
                                )
                    if len(osb_tiles) <= pair:
                        osb = o_p.tile([P, 2, NC_N, DOUT], f32, tag="osb")
                        osb_tiles.append(osb)
                    else:
                        osb = osb_tiles[pair]
                    for j in range(NC_N):
                        m_ap = auxf[:, MASKT0 + g * NC_N + j:
                                    MASKT0 + g * NC_N + j + 1]
                        z_in = z_tiles[j // 2][:, j % 2, :]
                        if (g // 2) % 2 == 0:
                            nc.scalar.activation(
                                out=osb[:, half, j, :], in_=z_in,
                                func=ActFn.Relu, bias=0.0, scale=m_ap,
                            )
                        else:
                            nc.vector.tensor_scalar(
                                out=osb[:, half, j, :], in0=z_in,
                                scalar1=m_ap, scalar2=0.0,
                                op0=Alu.mult, op1=Alu.max,
                            )
                    if half == 1:
                        nc.gpsimd.dma_start(
                            out=out_d[2 * pair:2 * pair + 2]
                                .rearrange("g (c p) e -> p g c e", p=P),
                            in_=osb,
                        )
    return nc


_CACHE = {}


def _get_nc():
    if "nc" not in _CACHE:
        _CACHE["nc"] = _build_nc()
    return _CACHE["nc"]


def kernel(x, adj, mask, weight, bias, gamma, beta):
    x = np.asarray(x, dtype=np.float32)
    adj = np.asarray(adj, dtype=np.float32)
    mask = np.asarray(mask, dtype=np.float32)
    weight = np.asarray(weight, dtype=np.float32)
    gamma = np.asarray(gamma, dtype=np.float32)
    beta = np.asarray(beta, dtype=np.float32)
    # bias cancels exactly in train-mode batchnorm (the mean absorbs it).

    n_tot = float(mask.sum())
    inv_n = np.float32(1.0 / n_tot)

    w_pack = weight.reshape(NC_D, P, DOUT).transpose(1, 0, 2).reshape(P, NC_D * DOUT)
    ident = np.eye(P, dtype=np.float32)
    gam = gamma.reshape(NC_E, P).T.copy()
    bet = beta.reshape(NC_E, P).T.copy()

    in_maps = []
    for c in range(NCORES):
        bs = slice(c * GPC, (c + 1) * GPC)
        adjm = adj[bs] * mask[bs][:, :, None]          # [GPC, n, m]
        adjT = adjm.transpose(0, 2, 1)                 # [GPC, m, n]
        blk_adj = adjT.reshape(GPC, NC_M, P, N)           # [g, kc, p, 512]
        blk_x = x[bs].reshape(GPC, NC_M, P, DIN)          # [g, kc, p, 256]
        blk = np.concatenate([blk_adj, blk_x], axis=3)    # [g, kc, p, 768]
        blob = blk.transpose(0, 2, 1, 3).reshape(GPC, P, GBLOB)

        maskT = mask[bs].reshape(GPC, NC_N, P).transpose(2, 0, 1) \
                        .reshape(P, GPC * NC_N)
        aux = np.empty((P, AUXW), dtype=np.float32)
        aux[:, W0:W0 + NC_D * DOUT] = w_pack
        aux[:, IDENT0:IDENT0 + P] = ident
        aux[:, ONES0:ONES0 + P] = 1.0
        aux[:, GAMMA0:GAMMA0 + NC_E] = gam
        aux[:, BETA0:BETA0 + NC_E] = bet
        aux[:, MASKT0:MASKT0 + GPC * NC_N] = maskT
        aux[:, INVN0] = inv_n
        aux[:, EPS0] = np.float32(EPS)
        in_maps.append(dict(blob=np.ascontiguousarray(blob, dtype=np.float32),
                            aux=np.ascontiguousarray(aux)))

    nc = _get_nc()
    res = run_bass_kernel_spmd(nc, in_maps, core_ids=list(range(NCORES)))
    out = np.concatenate([r["out"] for r in res.results], axis=0)
    return out.reshape(B, N, DOUT)
